# revision 41
# baseline (speedup 1.0000x reference)
"""DigitCapsule (dynamic routing) Trainium2 Bass kernel.

Problem: x (128,1152,8) f32, W (1,1152,10,16,8) f32 ->
  u_hat[b,r,o,do] = sum_di W[r,o,do,di] x[b,r,di]
  3 routing iterations (softmax over routes r, squash), output v (128,10,16,1).

Sharding: data-parallel over batch, 16 samples per core, W replicated.

Per-core layout (partition p = 16*j + b, j = r mod 8, b = batch-in-core):
  u[p, cc, do, o] = u_hat[b, 8*cc+j, o, do]   (fp16, 144 x 16 x 10 free)
u_hat is produced by 144 matmuls with a block-diagonal stationary operand
X_diag[(j,di)=64, (j',b)=128] (8 routes x 8 di contracted per matmul);
the zero-padded block-diagonal operand is built on the host and DMA'd in,
chunked so matmuls start as soon as the first chunk lands. Route-sums
(softmax denominator, s_j) are matmuls with a constant delta matrix
D[p,q] = (p%16 == q%16) that sums the 8 j-lanes per b and replicates the
result across all 128 partitions (so squash outputs are pre-broadcast for
the next elementwise pass). The do-contraction (agreement) is a pairwise
fp16 adder tree split across DVE and GpSimd. All Act functions used
(Square/Abs/Exp/Copy) live in one activation table set, so no reloads.
"""

import numpy as np

import concourse.bacc as bacc
import concourse.bass as bass
import concourse.tile as tile
from concourse import mybir
from concourse.bass_utils import run_bass_kernel_spmd

B, R, O, DO, DI = 128, 1152, 10, 16, 8
NCORES = 8
BC = B // NCORES          # 16 samples per core
J = 8                     # routes per matmul group
CC = R // J               # 144 matmul groups
OD = O * DO               # 160
F16 = mybir.dt.float16
F32 = mybir.dt.float32

PROD_BATCH = 3            # cc per production psum tile (one 2KB bank)
DMA_CHUNK = 24            # cc per input dma chunk
TB = 24                   # cc per agreement/premult batch
# gpsimd (Pool) offload: ~3.8x slower per element than DVE fp16 2x mode;
# it takes the trailing cc-slice of the agreement pass and the leading
# cc-slice of the premult pass (whose matmuls are streamed last).
AGR_POOL = 32
PRE_POOL = 36


def _squash(nc, pool, s_ps, v_out, scale):
    """v_out = squash(s_ps * scale), elementwise (reference semantics).

    With sb = s*scale: v = sb*|sb|/(1+sb^2), computed sqrt-free as
    s * |s*scale| * (scale/(1+(s*scale)^2)).  (The reference's 1e-9 eps
    inside its sqrt only matters for |sb| < 3e-5, where v ~ 0 anyway.)
    """
    P = s_ps.shape[0]
    m = pool.tile([P, DO, O], F32, tag="sq_m")
    a = pool.tile([P, DO, O], F32, tag="sq_a")
    d = pool.tile([P, DO, O], F32, tag="sq_d")
    p1 = pool.tile([P, DO, O], F32, tag="sq_p")
    nc.scalar.activation(m[:], s_ps[:], mybir.ActivationFunctionType.Square,
                         scale=scale)
    nc.scalar.activation(a[:], s_ps[:], mybir.ActivationFunctionType.Abs,
                         scale=scale)
    nc.vector.tensor_scalar(d[:], m[:], 1.0 / scale, 1.0 / scale,
                            mybir.AluOpType.mult, mybir.AluOpType.add)
    nc.vector.reciprocal(d[:], d[:])
    nc.vector.tensor_mul(p1[:], s_ps[:], a[:])
    nc.vector.tensor_mul(v_out[:], p1[:], d[:])


def _tree(nc, eng, pools, t, b_slice, accumulate, tag):
    """Reduce t [128, n, DO, O] over DO into b_slice [128, n, O] (fp16)."""
    l1p, l2p, l3p, l4p = pools
    n = t.shape[1]
    l1 = l1p.tile([128, n, 8, O], F16, tag=f"l1{tag}")
    eng.tensor_add(l1[:], t[:, :, 0:8, :], t[:, :, 8:16, :])
    l2 = l2p.tile([128, n, 4, O], F16, tag=f"l2{tag}")
    eng.tensor_add(l2[:], l1[:, :, 0:4, :], l1[:, :, 4:8, :])
    l3 = l3p.tile([128, n, 2, O], F16, tag=f"l3{tag}")
    eng.tensor_add(l3[:], l2[:, :, 0:2, :], l2[:, :, 2:4, :])
    if not accumulate:
        eng.tensor_add(b_slice, l3[:, :, 0, :], l3[:, :, 1, :])
    else:
        a4 = l4p.tile([128, n, O], F16, tag=f"l4{tag}")
        eng.tensor_add(a4[:], l3[:, :, 0, :], l3[:, :, 1, :])
        eng.tensor_add(b_slice, b_slice, a4[:])


def build_nc(reps=1):
    nc = bacc.Bacc("TRN2", debug=False)
    wt_d = nc.dram_tensor("wt", [64, CC, DO, O], F16, kind="ExternalInput")
    xd_d = nc.dram_tensor("xd", [64, CC, 128], F16, kind="ExternalInput")
    d16_d = nc.dram_tensor("d16", [128, 128], F16, kind="ExternalInput")
    d32_d = nc.dram_tensor("d32", [128, 128], F32, kind="ExternalInput")
    dout_d = nc.dram_tensor("dout", [128, BC], F16, kind="ExternalInput")
    out_d = nc.dram_tensor("out", [BC, O, DO], F32, kind="ExternalOutput")

    # agreement cc-ranges: DVE groups then the gpsimd slice
    dve_cc = CC - AGR_POOL
    agr_sls = []
    lo = 0
    while lo < dve_cc:
        agr_sls.append(slice(lo, min(lo + TB, dve_cc)))
        lo += TB
    gsl = slice(dve_cc, CC)
    npart = len(agr_sls) + 1
    # premult cc-ranges: gpsimd takes [0, PRE_POOL), DVE the rest
    pre_sls = []
    lo = PRE_POOL
    while lo < CC:
        pre_sls.append(slice(lo, min(lo + TB, CC)))
        lo += TB

    with tile.TileContext(nc) as tc:
        with (
            tc.tile_pool(name="const", bufs=1) as const,
            tc.tile_pool(name="prod", bufs=1) as prod,
            tc.tile_pool(name="main", bufs=1) as main,
            tc.tile_pool(name="sq", bufs=2) as sq,
            tc.tile_pool(name="tp", bufs=2) as tp,
            tc.tile_pool(name="l1p", bufs=2) as l1p,
            tc.tile_pool(name="l2p", bufs=2) as l2p,
            tc.tile_pool(name="l3p", bufs=2) as l3p,
            tc.tile_pool(name="l4p", bufs=2) as l4p,
            tc.tile_pool(name="gp", bufs=1) as gp,
            tc.tile_pool(name="pp", bufs=5, space=bass.MemorySpace.PSUM) as pp,
            tc.tile_pool(name="pss", bufs=1, space=bass.MemorySpace.PSUM) as pss,
            tc.tile_pool(name="psd", bufs=1, space=bass.MemorySpace.PSUM) as psd,
            tc.tile_pool(name="psw", bufs=1, space=bass.MemorySpace.PSUM) as psw,
        ):
            zero = const.tile([128, 1], F32)
            zero2 = const.tile([128, 1], F32)
            # memset on Act itself: exp's bias dep must not ride another
            # engine's completion counter (conservative position-at-emit
            # waits would serialize the exps behind that engine's queue)
            nc.scalar.memzero(zero[:])
            d16 = const.tile([128, 128], F16)
            d32 = const.tile([128, 128], F32)
            dout = const.tile([128, BC], F16)

            gpools = (l1p, l2p, l3p, l4p)

            for _rep in range(reps):
                wt = prod.tile([64, CC, DO, O], F16)
                xd = prod.tile([64, CC, 128], F16)
                wu_ps = psw.tile([16, 128], F32, tag="w")
                for ch in range(CC // DMA_CHUNK):
                    sl = slice(ch * DMA_CHUNK, (ch + 1) * DMA_CHUNK)
                    nc.sync.dma_start(xd[:, sl], xd_d[:, sl])
                    nc.sync.dma_start(wt[:, sl], wt_d[:, sl])
                    if ch == 0 and _rep == 0:
                        nc.sync.dma_start(d16[:], d16_d[:])
                    if ch == 2 and _rep == 0:
                        nc.sync.dma_start(d32[:], d32_d[:])
                        nc.sync.dma_start(dout[:], dout_d[:])
                    # keep-warm matmuls paced by chunk arrival: hold the PE
                    # p-state up through the load phase
                    nc.tensor.matmul(wu_ps[0:16, 0:64],
                                     xd[:, sl.start, 0:16],
                                     xd[:, sl.start, 0:64],
                                     start=True, stop=True)

                u = main.tile([128, CC, DO, O], F16)
                s_ps = pss.tile([128, DO, O], F32, tag="s")

                # ---- produce u_hat; fold in iter-0 route-sum (uniform c) ----
                nb = CC // PROD_BATCH
                gs_done = 0
                for g in range(nb):
                    ps = pp.tile([128, PROD_BATCH, OD], F32, tag="pp")
                    for i in range(PROD_BATCH):
                        cc = g * PROD_BATCH + i
                        nc.tensor.matmul(
                            ps[:, i, :], xd[:, cc, :], wt[:, cc, :, :],
                            start=True, stop=True,
                        )
                    sl = slice(g * PROD_BATCH, (g + 1) * PROD_BATCH)
                    src = ps[:].rearrange("p c (do o) -> p c do o", do=DO)
                    if g % 2 == 0:
                        nc.scalar.copy(u[:, sl, :, :], src)
                    else:
                        nc.vector.tensor_copy(u[:, sl, :, :], src)
                    # absorb iter-0 route-sum matmuls into the PE slack of
                    # the dma-paced load phase, one chunk behind the copies
                    if g % 8 == 7 and g >= 15:
                        hi = (g - 15) // 8 * 8 + 8
                        for gsx in range(gs_done, hi):
                            for i in range(PROD_BATCH):
                                cc = gsx * PROD_BATCH + i
                                nc.tensor.matmul(
                                    s_ps[:], d16[:], u[:, cc, :, :],
                                    start=(cc == 0), stop=(cc == CC - 1),
                                )
                        gs_done = hi
                for cc in range(gs_done * PROD_BATCH, CC):
                    nc.tensor.matmul(
                        s_ps[:], d16[:], u[:, cc, :, :],
                        start=(cc == 0), stop=(cc == CC - 1),
                    )

                v = main.tile([128, DO, O], F16)
                _squash(nc, sq, s_ps, v, 1.0 / R)

                # per-group b tiles: keeps DVE/gpsimd agreement chains and
                # the Act exps fully decoupled (per-tile dep tracking would
                # otherwise serialize them)
                b_g = [main.tile([128, s.stop - s.start, O], F16,
                                 name=f"bg{gi}")
                       for gi, s in enumerate(agr_sls)]
                b_p = main.tile([128, AGR_POOL, O], F16, name="bgp")
                e = main.tile([128, CC, O], F32)
                er_p = main.tile([128, npart, O], F32)
                e_r = main.tile([128, O], F32)
                inv = main.tile([128, O], F32)
                c16 = main.tile([128, CC, O], F16)

                for it in (1, 2):
                    final = it == 2
                    den = psd.tile([128, O], F32, tag="den")
                    # ---- agreement: b_ij (+)= sum_do u * v ----
                    # gpsimd takes the trailing cc-slice of the pass
                    tg = gp.tile([128, AGR_POOL, DO, O], F16, tag="tg")
                    ha = AGR_POOL // 2
                    v_bg = v[:].unsqueeze(1).broadcast_to((128, ha, DO, O))
                    nc.gpsimd.tensor_mul(tg[:, 0:ha], u[:, dve_cc:dve_cc + ha, :, :], v_bg)
                    _tree(nc, nc.gpsimd, gpools, tg[:, 0:ha], b_p[:, 0:ha, :],
                          it == 2, "g")
                    nc.gpsimd.tensor_mul(tg[:, ha:AGR_POOL],
                                         u[:, dve_cc + ha:CC, :, :], v_bg)
                    _tree(nc, nc.gpsimd, gpools, tg[:, ha:AGR_POOL],
                          b_p[:, ha:AGR_POOL, :], it == 2, "g2")
                    wt_ps = psw.tile([16, 128], F32, tag="w")
                    for gi, sl in enumerate(agr_sls):
                        n = sl.stop - sl.start
                        t = tp.tile([128, TB, DO, O], F16, tag="t")
                        v_b = v[:].unsqueeze(1).broadcast_to((128, n, DO, O))
                        nc.vector.tensor_mul(t[:, 0:n], u[:, sl, :, :], v_b)
                        l1 = l1p.tile([128, n, 8, O], F16, tag="l1v")
                        nc.vector.tensor_add(l1[:], t[:, 0:n, 0:8, :],
                                             t[:, 0:n, 8:16, :])
                        # keep-warm matmul paced by the tree: keeps the PE
                        # p-state ramped through the agreement phase
                        nc.tensor.matmul(wt_ps[0:10, 0:80], l1[:, 0, 0, :],
                                         l1[:, 0, :, :], start=True,
                                         stop=True)
                        l2 = l2p.tile([128, n, 4, O], F16, tag="l2v")
                        nc.vector.tensor_add(l2[:], l1[:, :, 0:4, :],
                                             l1[:, :, 4:8, :])
                        l3 = l3p.tile([128, n, 2, O], F16, tag="l3v")
                        nc.vector.tensor_add(l3[:], l2[:, :, 0:2, :],
                                             l2[:, :, 2:4, :])
                        nc.tensor.matmul(wt_ps[0:10, 0:20], l3[:, 0, 0, :],
                                         l3[:, 0, :, :], start=True,
                                         stop=True)
                        if it != 2:
                            nc.vector.tensor_add(b_g[gi][:], l3[:, :, 0, :],
                                                 l3[:, :, 1, :])
                        else:
                            a4 = l4p.tile([128, n, O], F16, tag="l4v")
                            nc.vector.tensor_add(a4[:], l3[:, :, 0, :],
                                                 l3[:, :, 1, :])
                            nc.vector.tensor_add(b_g[gi][:], b_g[gi][:], a4[:])
                        nc.scalar.activation(
                            e[:, sl, :], b_g[gi][:],
                            mybir.ActivationFunctionType.Exp, bias=zero[:])
                        nc.tensor.matmul(wt_ps[0:10, 0:O],
                                         e[:, sl.start, :],
                                         e[:, sl.start, :],
                                         start=True, stop=True)
                        if gi >= 1:
                            pv = agr_sls[gi - 1]
                            nv = pv.stop - pv.start
                            den_b = den[:].unsqueeze(1).broadcast_to(
                                (128, nv, O))
                            nc.tensor.matmul(
                                den_b, d32[:], e[:, pv, :],
                                start=(gi == 1), stop=False)
                    # zero-valued bias derived from the last DVE-group exp:
                    # forces exp(gsl) to schedule after the group exps, so
                    # those overlap the agreement instead of queueing behind
                    # exp(gsl)'s wait on the gpsimd chain
                    nc.vector.tensor_scalar(
                        zero2[:], e[:, agr_sls[-1].stop - 1, 0:1], 0.0, 0.0,
                        mybir.AluOpType.mult, mybir.AluOpType.mult)
                    nc.scalar.activation(
                        e[:, dve_cc:dve_cc + ha, :], b_p[:, 0:ha, :],
                        mybir.ActivationFunctionType.Exp, bias=zero2[:])
                    nc.scalar.activation(
                        e[:, dve_cc + ha:CC, :], b_p[:, ha:AGR_POOL, :],
                        mybir.ActivationFunctionType.Exp, bias=zero2[:])
                    lg = len(agr_sls) - 1
                    nlg = agr_sls[lg].stop - agr_sls[lg].start
                    nc.tensor.matmul(
                        den[:].unsqueeze(1).broadcast_to((128, nlg, O)),
                        d32[:], e[:, agr_sls[lg], :],
                        start=False, stop=False)
                    nc.tensor.matmul(
                        den[:].unsqueeze(1).broadcast_to((128, ha, O)),
                        d32[:], e[:, dve_cc:dve_cc + ha, :],
                        start=False, stop=False)
                    nc.tensor.matmul(
                        den[:].unsqueeze(1).broadcast_to((128, AGR_POOL - ha, O)),
                        d32[:], e[:, dve_cc + ha:CC, :],
                        start=False, stop=True)
                    nc.vector.reciprocal(inv[:], den[:])
                    inv_b2 = inv[:].unsqueeze(1)

                    # ---- s = sum_r c * u ----
                    sp_p = BC if final else 128
                    lhs = dout if final else d16
                    s_ps2 = pss.tile([sp_p, DO, O], F32, tag="s")
                    # gpsimd premultiplies the leading cc-slice; its matmuls
                    # are issued last so PE streams DVE-made tiles first.
                    nc.gpsimd.tensor_mul(
                        c16[:, 0:PRE_POOL, :], e[:, 0:PRE_POOL, :],
                        inv_b2.broadcast_to((128, PRE_POOL, O)))
                    t2g = gp.tile([128, PRE_POOL, DO, O], F16, tag="tg")
                    hp = PRE_POOL // 2
                    c_bg = c16[:, 0:hp, :].unsqueeze(2).broadcast_to(
                        (128, hp, DO, O))
                    nc.gpsimd.tensor_mul(t2g[:, 0:hp], u[:, 0:hp, :, :], c_bg)
                    c_bg2 = c16[:, hp:PRE_POOL, :].unsqueeze(2).broadcast_to(
                        (128, PRE_POOL - hp, DO, O))
                    nc.gpsimd.tensor_mul(t2g[:, hp:PRE_POOL],
                                         u[:, hp:PRE_POOL, :, :], c_bg2)
                    first_mm = True
                    for sl in pre_sls:
                        n = sl.stop - sl.start
                        nc.vector.tensor_mul(
                            c16[:, sl, :], e[:, sl, :],
                            inv_b2.broadcast_to((128, n, O)))
                        nc.tensor.matmul(wt_ps[0:10, 0:O],
                                         c16[:, sl.start, :],
                                         c16[:, sl.start, :],
                                         start=True, stop=True)
                        t = tp.tile([128, TB, DO, O], F16, tag="t")
                        c_b = c16[:, sl, :].unsqueeze(2).broadcast_to(
                            (128, n, DO, O))
                        nc.vector.tensor_mul(t[:, 0:n], u[:, sl, :, :], c_b)
                        for i in range(n):
                            nc.tensor.matmul(
                                s_ps2[:], lhs[:, :sp_p], t[:, i, :, :],
                                start=first_mm, stop=False,
                            )
                            first_mm = False
                    for i in range(PRE_POOL):
                        nc.tensor.matmul(
                            s_ps2[:], lhs[:, :sp_p], t2g[:, i, :, :],
                            start=False, stop=(i == PRE_POOL - 1),
                        )

                    if not final:
                        _squash(nc, sq, s_ps2, v, 1.0)
                    else:
                        v2 = main.tile([BC, DO, O], F32)
                        _squash(nc, sq, s_ps2, v2, 1.0)
                        v2p = main.tile([BC, O, DO], F32)
                        nc.vector.tensor_copy(v2p[:], v2[:].transpose((0, 2, 1)))
                        nc.sync.dma_start(out_d[:], v2p[:])

    nc.compile()
    return nc


_CACHE = {}


def _get_nc():
    if "nc" not in _CACHE:
        _CACHE["nc"] = build_nc()
    return _CACHE["nc"]


def _prep_const():
    if "const" not in _CACHE:
        p = np.arange(128)
        d16 = (p[:, None] % 16 == p[None, :] % 16).astype(np.float16)
        d32 = d16.astype(np.float32)
        dout = (p[:, None] % 16 == np.arange(BC)[None, :]).astype(np.float16)
        _CACHE["const"] = (d16, d32, dout)
    return _CACHE["const"]


def kernel(x: np.ndarray, W: np.ndarray) -> np.ndarray:
    x = np.asarray(x, dtype=np.float32)
    W = np.asarray(W, dtype=np.float32)
    nc = _get_nc()
    d16, d32, dout = _prep_const()
    W5 = np.ascontiguousarray(W.reshape(R, O, DO, DI))
    # wt[8j+di, cc, do, o] = W[8cc+j, o, do, di]
    wt = np.ascontiguousarray(
        W5.reshape(CC, J, O, DO, DI).transpose(1, 4, 0, 3, 2)
    ).reshape(64, CC, DO, O).astype(np.float16)
    in_maps = []
    for q in range(NCORES):
        xq = x[BC * q : BC * (q + 1)]           # [16, 1152, 8]
        xf = np.ascontiguousarray(
            xq.reshape(BC, CC, J, DI).transpose(2, 3, 1, 0)
        ).astype(np.float16)                     # [j, di, cc, b]
        # host-built block-diagonal stationary: xd[8j+di, cc, 16j'+b] =
        # xf[j, di, cc, b] if j == j' else 0
        xd = np.zeros((J, DI, CC, J, BC), dtype=np.float16)
        for j in range(J):
            xd[j, :, :, j, :] = xf[j]
        xd = np.ascontiguousarray(xd).reshape(64, CC, 128)
        in_maps.append({"wt": wt, "xd": xd, "d16": d16, "d32": d32,
                        "dout": dout})
    res = run_bass_kernel_spmd(nc, in_maps, core_ids=list(range(NCORES)))
    out = np.concatenate([res.results[q]["out"] for q in range(NCORES)], axis=0)
    return out.reshape(B, O, DO, 1).astype(np.float32)
